# revision 6
# baseline (speedup 1.0000x reference)
"""Trainium2 Bass kernel for nn_CustomConv2d_84018150244418.

Conv2d: x[16,128,128,128] (NCHW) * weff[256,128,3,3] (OIHW, weight scaled by
outer(kx,kx), kx=[0.7,1,0.7]) + bias, stride 1, pad 1 -> out[16,256,128,128].

Strategy: data-parallel over batch across 8 cores (2 images/core). Per core,
implicit GEMM: Cin=128 sits on the SBUF partition (contraction) dim; each of
the 9 filter taps is a [128cin x 128cout] stationary operand against a shifted
image window streamed 512 spatial positions (4 output rows) at a time,
accumulating all 9 taps in PSUM (fp32r matmuls, 1 cycle/row at N=512).
Bias-add is fused into the PSUM->SBUF eviction on the scalar engine.

The image lives in SBUF in 16-row bands padded to 130 columns with zeroed
borders, so every tap is a uniform strided window (no edge special-casing in
the matmul loop).
"""

import os
import sys

import numpy as np

sys.path.insert(0, "/opt/trn_rl_repo")

import concourse.bacc as bacc
import concourse.mybir as mybir
import concourse.tile as tile
from concourse.bass_utils import run_bass_kernel_spmd

N_CORES = 8
IMGS = 2          # images per core (16 / 8)
CIN = 128
COUT = 256
H = 128
W = 128
WP = W + 2        # zero-padded row width in SBUF
BANDS = 8         # bands per image
BAND_OUT = 16     # output rows per band
BAND_IN = BAND_OUT + 2
CROWS = 4         # output rows per matmul chunk (4*128 = 512 free dim)
CHUNKS = BAND_OUT // CROWS

DEN = 0.7
KX = (DEN, 1.0, DEN)

F32 = mybir.dt.float32
F32R = mybir.dt.float32r

_built = None


def _build():
    nc = bacc.Bacc("TRN2")
    x = nc.declare_dram_parameter("x", [IMGS, CIN, H, W], F32R, isOutput=False)
    w = nc.declare_dram_parameter("w", [CIN, 9, COUT], F32, isOutput=False)
    b = nc.declare_dram_parameter("b", [COUT, 1], F32, isOutput=False)
    out = nc.declare_dram_parameter("out", [IMGS, COUT, H, W], F32, isOutput=True)

    with tile.TileContext(nc) as tc:
        with (
            tc.tile_pool(name="const", bufs=1) as const,
            tc.tile_pool(name="bands", bufs=3) as bandp,
            tc.tile_pool(name="psum", bufs=4, space="PSUM") as psump,
            tc.tile_pool(name="outs", bufs=4) as outp,
        ):
            # Weights arrive host-pretransposed as [cin, tap, cout]; scale each
            # tap by its separable kx[i]*kx[j] factor on device.
            wraw = const.tile([CIN, 9, COUT], F32, tag="wraw")
            nc.sync.dma_start(wraw[:], w[:])
            wts = const.tile([CIN, 9, COUT], F32R, tag="wts")
            for t in range(9):
                s = float(KX[t // 3] * KX[t % 3])
                nc.scalar.mul(wts[:, t, :], wraw[:, t, :], s)

            bias_t = const.tile([128, 2], F32, tag="bias")
            for h2 in range(2):
                nc.sync.dma_start(bias_t[:, h2 : h2 + 1], b[h2 * 128 : (h2 + 1) * 128, :])

            # walrus rejects memset as a producer of fp32r-matmul inputs, so
            # pad zeros are written via DVE copy from this fp32 zeros tile.
            zsrc = const.tile([128, W], F32, tag="zsrc")
            nc.any.memset(zsrc[:], 0.0)

            for n in range(IMGS):
                for k in range(BANDS):
                    band = bandp.tile([CIN, BAND_IN, WP], F32R, tag="band")
                    # zero left/right padding columns
                    nc.vector.tensor_copy(band[:, :, 0], zsrc[:, :BAND_IN])
                    nc.vector.tensor_copy(band[:, :, WP - 1], zsrc[:, :BAND_IN])
                    if k == 0:
                        nc.vector.tensor_copy(band[:, 0, 1 : W + 1], zsrc[:, :W])
                        nc.sync.dma_start(
                            band[:, 1:BAND_IN, 1 : W + 1], x[n, :, 0 : BAND_IN - 1, :]
                        )
                    elif k == BANDS - 1:
                        nc.vector.tensor_copy(band[:, BAND_IN - 1, 1 : W + 1], zsrc[:, :W])
                        nc.sync.dma_start(
                            band[:, 0 : BAND_IN - 1, 1 : W + 1],
                            x[n, :, k * BAND_OUT - 1 : H, :],
                        )
                    else:
                        nc.sync.dma_start(
                            band[:, :, 1 : W + 1],
                            x[n, :, k * BAND_OUT - 1 : k * BAND_OUT + BAND_IN - 1, :],
                        )

                    for c in range(CHUNKS):
                        for h2 in range(2):
                            ps = psump.tile([128, CROWS, W], F32, tag="ps")
                            for t in range(9):
                                i, j = t // 3, t % 3
                                rhs = band[:, 4 * c + i : 4 * c + i + CROWS, j : j + W]
                                lhsT = wts[:, t, 128 * h2 : 128 * (h2 + 1)]
                                nc.tensor.matmul(
                                    ps[:],
                                    lhsT,
                                    rhs,
                                    start=(t == 0),
                                    stop=(t == 8),
                                )
                            ob = outp.tile([128, CROWS, W], F32, tag="ob")
                            nc.scalar.add(ob[:], ps[:], bias_t[:, h2 : h2 + 1])
                            y0 = k * BAND_OUT + 4 * c
                            nc.sync.dma_start(
                                out[n, 128 * h2 : 128 * (h2 + 1), y0 : y0 + CROWS, :],
                                ob[:],
                            )
    nc.finalize()  # Bacc.finalize runs the register-allocation/compile passes
    return nc


def _prep_inputs(x, weight, bias):
    x = np.ascontiguousarray(np.asarray(x, dtype=np.float32))
    weight = np.asarray(weight, dtype=np.float32)
    bias = np.asarray(bias, dtype=np.float32)
    # [O, I, 3, 3] -> [I, 3*3, O], contiguous
    wT = np.ascontiguousarray(weight.transpose(1, 2, 3, 0)).reshape(CIN, 9, COUT)
    b2 = np.ascontiguousarray(bias.reshape(COUT, 1))
    in_maps = [
        {"x": x[i * IMGS : (i + 1) * IMGS], "w": wT, "b": b2} for i in range(N_CORES)
    ]
    return in_maps


def _run(in_maps, **kwargs):
    global _built
    if _built is None:
        _built = _build()
    return run_bass_kernel_spmd(_built, in_maps, list(range(N_CORES)), **kwargs)


def kernel(x, weight, bias):
    res = _run(_prep_inputs(x, weight, bias))
    return np.concatenate([res.results[i]["out"] for i in range(N_CORES)], axis=0)


def kernel_timed(x, weight, bias, **kwargs):
    """Like kernel() but traces core 0 and returns (output, exec_time_ns)."""
    res = _run(_prep_inputs(x, weight, bias), trace=True, **kwargs)
    out = np.concatenate([res.results[i]["out"] for i in range(N_CORES)], axis=0)
    return out, res.exec_time_ns


# revision 13
# speedup vs baseline: 1.0379x; 1.0379x over previous
"""Trainium2 Bass kernel for nn_CustomConv2d_84018150244418.

Conv2d: x[16,128,128,128] (NCHW) * weff[256,128,3,3] (OIHW, weight scaled by
outer(kx,kx), kx=[0.7,1,0.7]) + bias, stride 1, pad 1 -> out[16,256,128,128].

Strategy: data-parallel over batch across 8 cores (2 images/core). Per core,
implicit GEMM: Cin=128 sits on the SBUF partition (contraction) dim; each of
the 9 filter taps is a [128cin x 128cout] stationary operand against a shifted
image window streamed 512 spatial positions (4 output rows) at a time,
accumulating all 9 taps in PSUM (fp32r matmuls, 1 cycle/row at N=512).
Bias-add is fused into the PSUM->SBUF eviction on the scalar engine.

The image lives in SBUF in 16-row bands padded to 130 columns with zeroed
borders, so every tap is a uniform strided window (no edge special-casing in
the matmul loop).
"""

import os
import sys

import numpy as np

sys.path.insert(0, "/opt/trn_rl_repo")

import concourse.bacc as bacc
import concourse.mybir as mybir
import concourse.tile as tile
from concourse.bass_utils import run_bass_kernel_spmd

N_CORES = 8
IMGS = 2          # images per core (16 / 8)
CIN = 128
COUT = 256
H = 128
W = 128
WP = W + 2        # zero-padded row width in SBUF
BANDS = 8         # bands per image
BAND_OUT = 16     # output rows per band
BAND_IN = BAND_OUT + 2
CROWS = 4         # output rows per matmul chunk (4*128 = 512 free dim)
CHUNKS = BAND_OUT // CROWS

DEN = 0.7
KX = (DEN, 1.0, DEN)

F32 = mybir.dt.float32
F32R = mybir.dt.float32r

_built = None

# pool buffer counts (overridable for sim sweeps)
BUFS = {"bands": 3, "psum": 4, "outs": 4}


def _build():
    nc = bacc.Bacc("TRN2")
    x = nc.declare_dram_parameter("x", [IMGS, CIN, H, W], F32R, isOutput=False)
    w = nc.declare_dram_parameter("w", [CIN, 9, COUT], F32, isOutput=False)
    b = nc.declare_dram_parameter("b", [COUT, 1], F32, isOutput=False)
    out = nc.declare_dram_parameter("out", [IMGS, COUT, H, W], F32, isOutput=True)

    with tile.TileContext(nc) as tc:
        with (
            tc.tile_pool(name="const", bufs=1) as const,
            tc.tile_pool(name="bands", bufs=BUFS["bands"]) as bandp,
            tc.tile_pool(name="psum", bufs=BUFS["psum"], space="PSUM") as psump,
            tc.tile_pool(name="outs", bufs=BUFS["outs"]) as outp,
        ):
            # Weights arrive host-pretransposed as [cin, tap, cout]; scale each
            # tap by its separable kx[i]*kx[j] factor on device.
            wraw = const.tile([CIN, 9, COUT], F32, tag="wraw")
            nc.scalar.dma_start(wraw[:], w[:])
            wts = const.tile([CIN, 9, COUT], F32R, tag="wts")
            for t in range(9):
                s = float(KX[t // 3] * KX[t % 3])
                nc.scalar.mul(wts[:, t, :], wraw[:, t, :], s)

            bias_t = const.tile([128, 2], F32, tag="bias")
            for h2 in range(2):
                nc.scalar.dma_start(bias_t[:, h2 : h2 + 1], b[h2 * 128 : (h2 + 1) * 128, :])

            # walrus rejects memset as a producer of fp32r-matmul inputs, so
            # pad zeros are written via DVE copy from this fp32 zeros tile.
            zsrc = const.tile([128, W], F32, tag="zsrc")
            nc.any.memset(zsrc[:], 0.0)

            for n in range(IMGS):
                for k in range(BANDS):
                    band = bandp.tile([CIN, BAND_IN, WP], F32R, tag="band")
                    # zero left/right padding columns
                    nc.vector.tensor_copy(band[:, :, 0], zsrc[:, :BAND_IN])
                    nc.vector.tensor_copy(band[:, :, WP - 1], zsrc[:, :BAND_IN])
                    if k == 0:
                        nc.vector.tensor_copy(band[:, 0, 1 : W + 1], zsrc[:, :W])
                        # split so chunk 0 (band rows 0..5) starts early
                        nc.sync.dma_start(
                            band[:, 1:6, 1 : W + 1], x[n, :, 0:5, :]
                        )
                        nc.sync.dma_start(
                            band[:, 6:BAND_IN, 1 : W + 1], x[n, :, 5 : BAND_IN - 1, :]
                        )
                    elif k == BANDS - 1:
                        nc.vector.tensor_copy(band[:, BAND_IN - 1, 1 : W + 1], zsrc[:, :W])
                        nc.sync.dma_start(
                            band[:, 0 : BAND_IN - 1, 1 : W + 1],
                            x[n, :, k * BAND_OUT - 1 : H, :],
                        )
                    else:
                        nc.sync.dma_start(
                            band[:, :, 1 : W + 1],
                            x[n, :, k * BAND_OUT - 1 : k * BAND_OUT + BAND_IN - 1, :],
                        )

                    for c in range(CHUNKS):
                        for h2 in range(2):
                            ps = psump.tile([128, CROWS, W], F32, tag="ps")
                            for t in range(9):
                                i, j = t // 3, t % 3
                                rhs = band[:, 4 * c + i : 4 * c + i + CROWS, j : j + W]
                                lhsT = wts[:, t, 128 * h2 : 128 * (h2 + 1)]
                                nc.tensor.matmul(
                                    ps[:],
                                    lhsT,
                                    rhs,
                                    start=(t == 0),
                                    stop=(t == 8),
                                )
                            ob = outp.tile([128, CROWS, W], F32, tag="ob")
                            nc.scalar.add(ob[:], ps[:], bias_t[:, h2 : h2 + 1])
                            y0 = k * BAND_OUT + 4 * c
                            # alternate stores across the two HWDGE queues
                            dma_eng = nc.sync if h2 == 0 else nc.scalar
                            dma_eng.dma_start(
                                out[n, 128 * h2 : 128 * (h2 + 1), y0 : y0 + CROWS, :],
                                ob[:],
                            )
    nc.finalize()  # Bacc.finalize runs the register-allocation/compile passes
    return nc


def _prep_inputs(x, weight, bias):
    x = np.ascontiguousarray(np.asarray(x, dtype=np.float32))
    weight = np.asarray(weight, dtype=np.float32)
    bias = np.asarray(bias, dtype=np.float32)
    # [O, I, 3, 3] -> [I, 3*3, O], contiguous
    wT = np.ascontiguousarray(weight.transpose(1, 2, 3, 0)).reshape(CIN, 9, COUT)
    b2 = np.ascontiguousarray(bias.reshape(COUT, 1))
    in_maps = [
        {"x": x[i * IMGS : (i + 1) * IMGS], "w": wT, "b": b2} for i in range(N_CORES)
    ]
    return in_maps


def _run(in_maps, **kwargs):
    global _built
    if _built is None:
        _built = _build()
    return run_bass_kernel_spmd(_built, in_maps, list(range(N_CORES)), **kwargs)


def kernel(x, weight, bias):
    res = _run(_prep_inputs(x, weight, bias))
    return np.concatenate([res.results[i]["out"] for i in range(N_CORES)], axis=0)


def kernel_timed(x, weight, bias, **kwargs):
    """Like kernel() but traces core 0 and returns (output, exec_time_ns)."""
    res = _run(_prep_inputs(x, weight, bias), trace=True, **kwargs)
    out = np.concatenate([res.results[i]["out"] for i in range(N_CORES)], axis=0)
    return out, res.exec_time_ns
